# revision 1
# baseline (speedup 1.0000x reference)
"""Circular shift kernel for Trainium2 (Bass), SPMD over 8 NeuronCores.

Reference semantics: out = vec @ roll(eye(d), -1, axis=0), which is exactly
out[b, j] = vec[b, (j-1) mod d]  (a roll by +1 along the last axis).

Sharding: data-parallel along the batch axis — each of the 8 cores handles a
contiguous [1024, 4096] row block and performs the column roll locally with
direct DRAM->DRAM DMA copies (no SBUF bounce: each byte passes through an
SDMA engine once, so D2D sustains ~670 GB/s combined read+write per core
vs ~435 GB/s through SBUF).

Three DMAs per core, all on the SP HWDGE ring:
  bulk tail:  out_flat[4096:] = in_flat[4095:-1]  -- dst starts at the row-1
              boundary, so the 64-KiB descriptor cuts are all HBM-atom
              aligned (no partial-atom sharing between concurrent engines)
  bulk head:  out[0, 1:] = in[0, :-1]             -- one 16380-B descriptor
  wrap:       out[:, 0] = in[:, 4095]             -- 1024 x 4-B descriptors,
              serialized AFTER the bulk: sub-512-B HBM writes are
              read-modify-write on the surrounding granule, so they must not
              run concurrently with bulk writes to adjacent bytes.
"""

import numpy as np

N_CORES = 8
ROWS = 8192
COLS = 4096
SHARD_ROWS = ROWS // N_CORES  # 1024
N = SHARD_ROWS * COLS  # elems per shard


def _build_nc():
    import concourse.bass as bass
    import concourse.mybir as mybir

    nc = bass.Bass("TRN2", monotonic_sem_count=0, enable_partition_id=False)
    x = nc.dram_tensor(
        "vec", [SHARD_ROWS, COLS], mybir.dt.float32, kind="ExternalInput"
    )
    y = nc.dram_tensor(
        "out", [SHARD_ROWS, COLS], mybir.dt.float32, kind="ExternalOutput"
    )
    xf = x[:, :].flatten()
    yf = y[:, :].flatten()

    with nc.semaphore("dma_done") as sem:
        nc.sync.dma_start(out=yf[COLS:N], in_=xf[COLS - 1 : N - 1]).then_inc(sem, 16)
        nc.sync.dma_start(out=yf[1:COLS], in_=xf[0 : COLS - 1]).then_inc(sem, 16)
        nc.sync.wait_ge(sem, 32)
        with nc.allow_non_contiguous_dma(reason="wrap column: 1 elem per row"):
            nc.sync.dma_start(out=y[:, 0:1], in_=x[:, COLS - 1 : COLS]).then_inc(
                sem, 16
            )
        nc.sync.wait_ge(sem, 48)
    return nc


def run(vec: np.ndarray, **spmd_kwargs):
    """Build + run the SPMD kernel; returns (full_output, BassKernelResults)."""
    from concourse import bass_utils

    vec = np.ascontiguousarray(vec, dtype=np.float32)
    assert vec.shape == (ROWS, COLS), vec.shape
    nc = _build_nc()
    in_maps = [
        {"vec": vec[i * SHARD_ROWS : (i + 1) * SHARD_ROWS]} for i in range(N_CORES)
    ]
    res = bass_utils.run_bass_kernel_spmd(
        nc, in_maps, core_ids=list(range(N_CORES)), **spmd_kwargs
    )
    out = np.concatenate([r["out"] for r in res.results], axis=0)
    return out, res


def kernel(vec: np.ndarray) -> np.ndarray:
    out, _ = run(vec)
    return out



# revision 2
# speedup vs baseline: 1.0663x; 1.0663x over previous
"""Circular shift kernel V1: granule-split, fully-concurrent DMAs.

out[b, j] = in[b, (j-1) mod 4096]  (roll by +1 along the last axis).

Output rows are split at the 512-B HBM granule boundary (128 f32 cols):
  - bulk (SP HWDGE ring):   out[:, 128:4096] <- in[:, 127:4095]
      strided D2D, 1024 x 15872-B descriptors; every write is 512-B
      granule aligned and never touches granule 0 of any row.
  - head block (ACT ring):  out[:, 0:128] composed in SBUF
      load A: C[:, :, 1:128] <- in[:, 0:127]
      load B: C[:, :, 0:1]   <- in[:, 4095]     (4-B strided reads: safe)
      store:  out[:, 0:128]  <- C               (1024 x 512-B aligned writes)
The bulk and the head-block chain write disjoint 512-B granules, so they
run fully concurrently -- no write-after-write serialization like the
baseline's flat-bulk + wrap-column design.
"""

import numpy as np

N_CORES = 8
ROWS = 8192
COLS = 4096
SHARD_ROWS = ROWS // N_CORES  # 1024
G = 128  # f32 elements per 512-B HBM granule


def _build_nc():
    import concourse.bass as bass
    import concourse.mybir as mybir

    nc = bass.Bass("TRN2", monotonic_sem_count=0, enable_partition_id=False)
    x = nc.dram_tensor(
        "vec", [SHARD_ROWS, COLS], mybir.dt.float32, kind="ExternalInput"
    )
    y = nc.dram_tensor(
        "out", [SHARD_ROWS, COLS], mybir.dt.float32, kind="ExternalOutput"
    )

    P = 128
    Q = SHARD_ROWS // P  # 8

    with (
        nc.sbuf_tensor([P, Q * G], mybir.dt.float32) as C,
        nc.semaphore("s_load") as sL,
        nc.semaphore("s_bulk") as sB,
        nc.semaphore("s_store") as sS,
    ):
        # bulk first so the big transfer starts as early as possible
        nc.sync.dma_start(out=y[:, G:COLS], in_=x[:, G - 1 : COLS - 1]).then_inc(
            sB, 16
        )

        C3 = C[:, :].rearrange("p (q c) -> p q c", c=G)
        a_src = x[:, 0 : G - 1].rearrange("(p q) c -> p q c", p=P)
        nc.scalar.dma_start(out=C3[:, :, 1:G], in_=a_src).then_inc(sL, 16)
        with nc.allow_non_contiguous_dma(reason="wrap column: 1 elem per row"):
            b_src = x[:, COLS - 1 : COLS].rearrange("(p q) c -> p q c", p=P)
            nc.scalar.dma_start(out=C3[:, :, 0:1], in_=b_src).then_inc(sL, 16)
        nc.scalar.wait_ge(sL, 32)
        head_dst = y[:, 0:G].rearrange("(p q) c -> p q c", p=P)
        nc.scalar.dma_start(out=head_dst, in_=C3).then_inc(sS, 16)

        nc.sync.wait_ge(sB, 16)
        nc.scalar.wait_ge(sS, 16)
    return nc


def run(vec: np.ndarray, **spmd_kwargs):
    """Build + run the SPMD kernel; returns (full_output, BassKernelResults)."""
    from concourse import bass_utils

    vec = np.ascontiguousarray(vec, dtype=np.float32)
    assert vec.shape == (ROWS, COLS), vec.shape
    nc = _build_nc()
    in_maps = [
        {"vec": vec[i * SHARD_ROWS : (i + 1) * SHARD_ROWS]} for i in range(N_CORES)
    ]
    res = bass_utils.run_bass_kernel_spmd(
        nc, in_maps, core_ids=list(range(N_CORES)), **spmd_kwargs
    )
    out = np.concatenate([r["out"] for r in res.results], axis=0)
    return out, res


def kernel(vec: np.ndarray) -> np.ndarray:
    out, _ = run(vec)
    return out
